# revision 2
# baseline (speedup 1.0000x reference)
"""Local (windowed) attention with rotary embeddings on 8 TRN2 NeuronCores, v2.

Problem: B=4 H=8 N=4096 D=64, window=128, look_backward=1 (j=256 keys/window),
rotary (position-in-context), causal+pad mask, softmax, PV.

v2 redesign (vs the transpose-based v1 at 106.7us):
  - d-major layout: q/k stored [d, t] so QK contracts d on PE partitions
    directly -- ZERO PE transposes (v1 spent 41us there).
  - fp16 everywhere off the PSUM path: matmuls run 1 cycle/col (4x over
    fp32), DVE elementwise gets the 2x_1p mode.
  - Rows pair-packed on partitions: rotary DVE/Pool ops run at full width;
    per-row matmul operands use partition-base 0/64 (sim-validated).
  - Rotate-half shift is a partition rotation in d-major, which no
    compute engine can do cheaply -- host supplies pre-shifted copies
    (qs/ks) via DMA instead.
  - Causal mask = one extra fp16 matmul (constant -30000 strict-upper
    matrix x identity) accumulated into the QK PSUM group.
  - Per-window rotary tables (angles i, i+128 for q / i for k) with the
    A/B relative-rotation trick: chunk c's keys serve windows c (own, qA)
    and c+1 (prev, qB).  Tables are [128,128] broadcast over windows.
  - PV in fp16 with a ones-column on v producing the softmax denominator;
    normalize = DVE reciprocal + Pool broadcast-multiply; fp16 output.

Cost-model budget per core (4 rows): DMA ~37us (serial bus, the wall),
ACT exp ~33us, DVE ~28us, PE ~27us, Pool ~27us.
"""

import numpy as np

import concourse.bass as bass
import concourse.bacc as bacc
import concourse.tile as tile
from concourse import mybir
from concourse.bass_utils import run_bass_kernel_spmd

B, H, N, D = 4, 8, 4096, 64
WIN = 128
NW = N // WIN            # 32 windows
NCORES = 8
ROWS = B * H             # 32 packed batch rows
RPC = ROWS // NCORES     # 4 rows per core
NPAIR = RPC // 2         # 2 row-pairs per core
ROPE = 10000.0
SCALE = D ** -0.5        # folded into q tables
MASKVAL = -30000.0
NSEG = 4                 # rotary / load segmentation
SEGC = N // NSEG         # 1024 cols per segment

F32 = mybir.dt.float32
F16 = mybir.dt.float16


def _tables():
    """Per-window rotary tables, [128, WIN] each (d' on partitions)."""
    f = np.arange(32, dtype=np.float64)
    omg = ROPE ** (-f / 32.0)                     # [32]
    i = np.arange(WIN, dtype=np.float64)

    def pair(ang_pos, scale):
        ang = ang_pos[None, :] * omg[:, None]     # [32, WIN]
        c64 = np.concatenate([np.cos(ang), np.cos(ang)], axis=0) * scale
        s64 = np.concatenate([-np.sin(ang), np.sin(ang)], axis=0) * scale
        # duplicate for the two packed rows
        return (np.concatenate([c64, c64], axis=0).astype(np.float16),
                np.concatenate([s64, s64], axis=0).astype(np.float16))

    cqa, sqa = pair(i, SCALE)          # q vs own chunk   (angle i)
    cqb, sqb = pair(i + WIN, SCALE)    # q vs prev chunk  (angle i+128)
    ck, sk = pair(i, 1.0)              # k                (angle jj')
    return dict(cqa=cqa, sqa=sqa, cqb=cqb, sqb=sqb, ck=ck, sk=sk)


TAB_NAMES = ["cqa", "sqa", "cqb", "sqb", "ck", "sk"]


def build_bass():
    nc = bacc.Bacc("TRN2", target_bir_lowering=False)
    qp_d = nc.declare_dram_parameter("qp", [NPAIR, 128, N], F16, isOutput=False)
    qs_d = nc.declare_dram_parameter("qs", [NPAIR, 128, N], F16, isOutput=False)
    kp_d = nc.declare_dram_parameter("kp", [NPAIR, 128, N], F16, isOutput=False)
    ks_d = nc.declare_dram_parameter("ks", [NPAIR, 128, N], F16, isOutput=False)
    vo_d = nc.declare_dram_parameter("vo", [RPC, WIN, NW, D], F16, isOutput=False)
    consts_d = nc.declare_dram_parameter("consts", [128, 8 * WIN], F16,
                                         isOutput=False)
    o_d = nc.declare_dram_parameter("o", [RPC, WIN, NW, D], F16, isOutput=True)

    with tile.TileContext(nc) as tc:
        with (
            tc.tile_pool(name="consts", bufs=1) as consts,
            tc.tile_pool(name="data", bufs=2) as data,
            tc.tile_pool(name="qab", bufs=2) as qabp,
            tc.tile_pool(name="tmp", bufs=1) as tmpp,
            tc.tile_pool(name="vop", bufs=2) as vop,
            tc.tile_pool(name="expp", bufs=5) as expp,
            tc.tile_pool(name="outp", bufs=2) as outp,
            tc.tile_pool(name="recp", bufs=2) as recp,
            tc.tile_pool(name="psim", bufs=2, space="PSUM") as psim,
            tc.tile_pool(name="ppo", bufs=1, space="PSUM") as ppo,
            tc.tile_pool(name="pdn", bufs=1, space="PSUM") as pdn,
        ):
            ctile = consts.tile([128, 8 * WIN], F16, tag="consts")
            nc.sync.dma_start(out=ctile, in_=consts_d[:, :])
            tab = {t: ctile[:, i * WIN:(i + 1) * WIN]
                   for i, t in enumerate(TAB_NAMES)}
            maskt = ctile[:, 6 * WIN:7 * WIN]
            ident = ctile[:, 7 * WIN:8 * WIN]
            ones = consts.tile([WIN, 1], F16, tag="ones")
            nc.vector.memset(ones, 1.0)

            def bcast(t, nwin):
                # [128, WIN] table -> [128, nwin, WIN] window-broadcast view
                return bass.AP(
                    tensor=t.tensor, offset=t.offset,
                    ap=[list(t.ap[0]), [0, nwin], list(t.ap[1])],
                )

            for p in range(NPAIR):
                qp = data.tile([128, N], F16, tag="qp")
                qs = data.tile([128, N], F16, tag="qs")
                kp = data.tile([128, N], F16, tag="kp")
                ks = data.tile([128, N], F16, tag="ks")
                # qAB: slot (c,0)=qrotA win c, slot (c,1)=qrotB win c+1
                qab = qabp.tile([128, NW, 2, WIN], F16, tag="qab")
                ta = tmpp.tile([128, N], F16, tag="ta")
                tb = tmpp.tile([128, N], F16, tag="tb")
                vo0 = vop.tile([WIN, NW, D], F16, tag="vo0")
                vo1 = vop.tile([WIN, NW, D], F16, tag="vo1")
                vo = [vo0, vo1]
                out0 = outp.tile([WIN, NW, D], F16, tag="out0")
                out1 = outp.tile([WIN, NW, D], F16, tag="out1")
                outr = [out0, out1]

                # ---- loads, segment-interleaved; small leading segments
                # so the first QK groups start ~5us instead of ~9
                SEGS = [(0, 1024), (1024, 2048), (2048, 3072), (3072, 4096)]
                for s, (lo, hi) in enumerate(SEGS):
                    sl = slice(lo, hi)
                    nc.sync.dma_start(out=kp[:, sl], in_=kp_d[p][:, sl])
                    nc.sync.dma_start(out=ks[:, sl], in_=ks_d[p][:, sl])
                    nc.sync.dma_start(out=qp[:, sl], in_=qp_d[p][:, sl])
                    nc.sync.dma_start(out=qs[:, sl], in_=qs_d[p][:, sl])
                    if s == 0:
                        nc.sync.dma_start(out=vo[0][:, 0:16, :],
                                          in_=vo_d[2 * p][:, 0:16, :])
                    elif s == 1:
                        nc.sync.dma_start(out=vo[1][:, 0:16, :],
                                          in_=vo_d[2 * p + 1][:, 0:16, :])
                        nc.sync.dma_start(out=vo[0][:, 16:NW, :],
                                          in_=vo_d[2 * p][:, 16:NW, :])
                    elif s == 2:
                        nc.sync.dma_start(out=vo[1][:, 16:NW, :],
                                          in_=vo_d[2 * p + 1][:, 16:NW, :])
                # qB slot of window NW is unused -> zero once
                nc.vector.memset(qab[:, NW - 1, 1, :], 0.0)

                # ---- rotary, segmented; d' on partitions so all ops are
                # full-width.  q -> qab slots (A: angle i vs own chunk,
                # B: angle i+128 vs prev chunk); k in-place into kp.
                # Coarser segmentation than the loads: DVE/Pool per-op
                # overhead is 60-190ns, so fewer, bigger ops win.

                def rot_b(wlo, whi):
                    # qB for windows [wlo, whi) -> slots (w-1, 1).
                    # cos product via tb, sin product direct into the slot,
                    # add on Pool.
                    n = whi - wlo
                    if n <= 0:
                        return
                    csl = slice(wlo * WIN, whi * WIN)
                    tbv = tb[:, csl].rearrange("p (w i) -> p w i", i=WIN)
                    nc.vector.tensor_mul(
                        tbv, qp[:, csl].rearrange("p (w i) -> p w i", i=WIN),
                        bcast(tab["cqb"], n))
                    nc.vector.tensor_mul(
                        qab[:, wlo - 1:whi - 1, 1, :],
                        qs[:, csl].rearrange("p (w i) -> p w i", i=WIN),
                        bcast(tab["sqb"], n))
                    nc.gpsimd.tensor_tensor(
                        out=qab[:, wlo - 1:whi - 1, 1, :],
                        in0=qab[:, wlo - 1:whi - 1, 1, :],
                        in1=tbv, op=mybir.AluOpType.add)

                for s, (lo, hi) in enumerate(SEGS):
                    sl = slice(lo, hi)
                    w0 = lo // WIN
                    WSEG = (hi - lo) // WIN
                    qpv = qp[:, sl].rearrange("p (w i) -> p w i", i=WIN)
                    qsv = qs[:, sl].rearrange("p (w i) -> p w i", i=WIN)
                    # boundary first: slot (w0-1, 1) = qB win w0 unblocks the
                    # previous segment's last QK group as soon as this
                    # segment's loads land (instead of after its full rotary)
                    if s > 0:
                        rot_b(w0, w0 + 1)
                    # k: krot in-place into kp (QK lhsT next-most urgent)
                    kpv = kp[:, sl].rearrange("p (w i) -> p w i", i=WIN)
                    ksv = ks[:, sl].rearrange("p (w i) -> p w i", i=WIN)
                    nc.vector.tensor_mul(kpv, kpv, bcast(tab["ck"], WSEG))
                    nc.gpsimd.tensor_mul(ksv, ksv, bcast(tab["sk"], WSEG))
                    nc.gpsimd.tensor_tensor(out=kpv, in0=kpv, in1=ksv,
                                            op=mybir.AluOpType.add)
                    # A variant: windows w0..w0+WSEG-1 -> slots (w, 0)
                    nc.vector.tensor_mul(
                        ta[:, sl].rearrange("p (w i) -> p w i", i=WIN),
                        qpv, bcast(tab["cqa"], WSEG))
                    nc.vector.tensor_mul(
                        qab[:, w0:w0 + WSEG, 0, :], qsv, bcast(tab["sqa"], WSEG))
                    nc.gpsimd.tensor_tensor(
                        out=qab[:, w0:w0 + WSEG, 0, :],
                        in0=qab[:, w0:w0 + WSEG, 0, :],
                        in1=ta[:, sl].rearrange("p (w i) -> p w i", i=WIN),
                        op=mybir.AluOpType.add)
                    # B variant for this segment's windows (minus boundary)
                    rot_b(w0 + 1 if s > 0 else 1, w0 + WSEG)

                # ---- chunk pipeline, ROW-INTERLEAVED: PE executes its queue
                # in order, so issuing row 1's early groups after row 0's
                # late ones would head-of-line block on late load segments.
                prev_expt = {0: None, 1: None}
                po = {}
                dn = {}
                for g in range(NW // 4):
                    for j in (0, 1):
                        b0 = 64 * j
                        sim = psim.tile([128, 4, 2 * WIN], F32)
                        for cc in range(4):
                            c = 4 * g + cc
                            nc.tensor.matmul(
                                sim[:, cc, :],
                                lhsT=kp[b0:b0 + 64, c * WIN:(c + 1) * WIN],
                                rhs=qab[b0:b0 + 64, c, :, :],
                                start=True, stop=False)
                            nc.tensor.matmul(
                                sim[:, cc, 0:WIN], lhsT=maskt, rhs=ident,
                                start=False, stop=True)
                        expt = expp.tile([128, 4, 2 * WIN], F16, tag="expt")
                        nc.scalar.activation(
                            out=expt, in_=sim,
                            func=mybir.ActivationFunctionType.Exp)
                        if g % 2 == 0:
                            # po slots are exactly 256B so 8 windows fill one
                            # PSUM bank -- a 65-col slot would cross the 2KB
                            # bank boundary and corrupt accumulation.  The
                            # softmax denominator comes from separate
                            # near-free ap_size=1 matmuls into dn.
                            po[j] = ppo.tile([128, 8, D], F32, tag=f"po{j}", name=f"po{j}")
                            dn[j] = pdn.tile([128, 8], F32, tag=f"dn{j}", name=f"dn{j}")
                        for cc in range(4):
                            w = 4 * g + cc
                            slot = po[j][:, w % 8, :]
                            dslot = dn[j][:, w % 8 : w % 8 + 1]
                            own = expt[:, cc, 0:WIN]
                            if w == 0:
                                nc.tensor.matmul(
                                    slot, lhsT=own, rhs=vo[j][:, 0, :],
                                    start=True, stop=True)
                                nc.tensor.matmul(
                                    dslot, lhsT=own, rhs=ones,
                                    start=True, stop=True)
                            else:
                                pt = expt if cc > 0 else prev_expt[j]
                                prev = pt[:, (w - 1) % 4, WIN:2 * WIN]
                                nc.tensor.matmul(
                                    slot, lhsT=prev, rhs=vo[j][:, w - 1, :],
                                    start=True, stop=False)
                                nc.tensor.matmul(
                                    slot, lhsT=own, rhs=vo[j][:, w, :],
                                    start=False, stop=True)
                                nc.tensor.matmul(
                                    dslot, lhsT=prev, rhs=ones,
                                    start=True, stop=False)
                                nc.tensor.matmul(
                                    dslot, lhsT=own, rhs=ones,
                                    start=False, stop=True)
                        prev_expt[j] = expt
                        if g % 2 == 1:
                            w0 = 8 * (g // 2)
                            rec = recp.tile([128, 8], F32, tag="rec")
                            nc.vector.reciprocal(rec, dn[j])
                            rb = bass.AP(
                                tensor=rec.tensor, offset=rec.offset,
                                ap=[list(rec.ap[0]), [1, 8], [0, D]])
                            # GPSIMD cannot read PSUM -> normalize on DVE
                            nc.vector.tensor_tensor(
                                out=outr[j][:, w0:w0 + 8, :],
                                in0=po[j], in1=rb,
                                op=mybir.AluOpType.mult)
                for j in (0, 1):
                    nc.sync.dma_start(out=o_d[2 * p + j], in_=outr[j])

    nc.compile()
    return nc


_NC_CACHE = None


def _get_nc():
    global _NC_CACHE
    if _NC_CACHE is None:
        _NC_CACHE = build_bass()
    return _NC_CACHE


def _in_maps(q, k, v):
    q_ = np.asarray(q, dtype=np.float32).reshape(ROWS, N, D)
    k_ = np.asarray(k, dtype=np.float32).reshape(ROWS, N, D)
    v_ = np.asarray(v, dtype=np.float32).reshape(ROWS, N, D)
    tabs = _tables()
    maskt = np.triu(np.full((WIN, WIN), MASKVAL, dtype=np.float32), 1).astype(np.float16)
    ident = np.eye(WIN, dtype=np.float16)
    consts = np.concatenate(
        [tabs[t] for t in TAB_NAMES] + [maskt, ident], axis=1)

    maps = []
    for c in range(NCORES):
        m = {"consts": consts}
        qp = np.empty((NPAIR, 128, N), np.float16)
        qsh = np.empty((NPAIR, 128, N), np.float16)
        kp = np.empty((NPAIR, 128, N), np.float16)
        ksh = np.empty((NPAIR, 128, N), np.float16)
        vo = np.empty((RPC, WIN, NW, D), np.float16)
        for p in range(NPAIR):
            for j in (0, 1):
                r = c * RPC + 2 * p + j
                qdm = q_[r].T            # [64, N]
                kdm = k_[r].T
                sl = slice(64 * j, 64 * j + 64)
                qp[p, sl] = qdm
                qsh[p, sl] = np.roll(qdm, -32, axis=0)
                kp[p, sl] = kdm
                ksh[p, sl] = np.roll(kdm, -32, axis=0)
        for rr in range(RPC):
            r = c * RPC + rr
            vv = v_[r].reshape(NW, WIN, D).transpose(1, 0, 2)  # [WIN, NW, D]
            vo[rr] = vv
        m.update(qp=qp, qs=qsh, kp=kp, ks=ksh, vo=vo)
        maps.append(m)
    return maps


def _run(q, k, v, **kw):
    nc = _get_nc()
    res = run_bass_kernel_spmd(nc, _in_maps(q, k, v), list(range(NCORES)), **kw)
    outs = []
    for c in range(NCORES):
        o = res.results[c]["o"].astype(np.float32)   # [RPC, WIN, NW, D]
        outs.append(o.transpose(0, 2, 1, 3).reshape(RPC, N, D))
    out = np.concatenate(outs, axis=0).reshape(B, H, N, D)
    return np.ascontiguousarray(out), res


def kernel(q, k, v):
    out, _ = _run(q, k, v)
    return out


# revision 3
# speedup vs baseline: 1.0043x; 1.0043x over previous
"""Local (windowed) attention with rotary embeddings on 8 TRN2 NeuronCores, v2.

Problem: B=4 H=8 N=4096 D=64, window=128, look_backward=1 (j=256 keys/window),
rotary (position-in-context), causal+pad mask, softmax, PV.

v2 redesign (vs the transpose-based v1 at 106.7us):
  - d-major layout: q/k stored [d, t] so QK contracts d on PE partitions
    directly -- ZERO PE transposes (v1 spent 41us there).
  - fp16 everywhere off the PSUM path: matmuls run 1 cycle/col (4x over
    fp32), DVE elementwise gets the 2x_1p mode.
  - Rows pair-packed on partitions: rotary DVE/Pool ops run at full width;
    per-row matmul operands use partition-base 0/64 (sim-validated).
  - Rotate-half shift is a partition rotation in d-major, which no
    compute engine can do cheaply -- host supplies pre-shifted copies
    (qs/ks) via DMA instead.
  - Causal mask = one extra fp16 matmul (constant -30000 strict-upper
    matrix x identity) accumulated into the QK PSUM group.
  - Per-window rotary tables (angles i, i+128 for q / i for k) with the
    A/B relative-rotation trick: chunk c's keys serve windows c (own, qA)
    and c+1 (prev, qB).  Tables are [128,128] broadcast over windows.
  - PV in fp16 with a ones-column on v producing the softmax denominator;
    normalize = DVE reciprocal + Pool broadcast-multiply; fp16 output.

Cost-model budget per core (4 rows): DMA ~37us (serial bus, the wall),
ACT exp ~33us, DVE ~28us, PE ~27us, Pool ~27us.
"""

import numpy as np

import concourse.bass as bass
import concourse.bacc as bacc
import concourse.tile as tile
from concourse import mybir
from concourse.bass_utils import run_bass_kernel_spmd

B, H, N, D = 4, 8, 4096, 64
WIN = 128
NW = N // WIN            # 32 windows
NCORES = 8
ROWS = B * H             # 32 packed batch rows
RPC = ROWS // NCORES     # 4 rows per core
NPAIR = RPC // 2         # 2 row-pairs per core
ROPE = 10000.0
SCALE = D ** -0.5        # folded into q tables
MASKVAL = -30000.0
NSEG = 4                 # rotary / load segmentation
SEGC = N // NSEG         # 1024 cols per segment

F32 = mybir.dt.float32
F16 = mybir.dt.float16


def _tables():
    """Per-window rotary tables, [128, WIN] each (d' on partitions)."""
    f = np.arange(32, dtype=np.float64)
    omg = ROPE ** (-f / 32.0)                     # [32]
    i = np.arange(WIN, dtype=np.float64)

    def pair(ang_pos, scale):
        ang = ang_pos[None, :] * omg[:, None]     # [32, WIN]
        c64 = np.concatenate([np.cos(ang), np.cos(ang)], axis=0) * scale
        s64 = np.concatenate([-np.sin(ang), np.sin(ang)], axis=0) * scale
        # duplicate for the two packed rows
        return (np.concatenate([c64, c64], axis=0).astype(np.float16),
                np.concatenate([s64, s64], axis=0).astype(np.float16))

    cqa, sqa = pair(i, SCALE)          # q vs own chunk   (angle i)
    cqb, sqb = pair(i + WIN, SCALE)    # q vs prev chunk  (angle i+128)
    ck, sk = pair(i, 1.0)              # k                (angle jj')
    return dict(cqa=cqa, sqa=sqa, cqb=cqb, sqb=sqb, ck=ck, sk=sk)


TAB_NAMES = ["cqa", "sqa", "cqb", "sqb", "ck", "sk"]


def build_bass():
    nc = bacc.Bacc("TRN2", target_bir_lowering=False)
    qp_d = nc.declare_dram_parameter("qp", [NPAIR, 128, N], F16, isOutput=False)
    qs_d = nc.declare_dram_parameter("qs", [NPAIR, 128, N], F16, isOutput=False)
    kp_d = nc.declare_dram_parameter("kp", [NPAIR, 128, N], F16, isOutput=False)
    ks_d = nc.declare_dram_parameter("ks", [NPAIR, 128, N], F16, isOutput=False)
    vo_d = nc.declare_dram_parameter("vo", [RPC, WIN, NW, D], F16, isOutput=False)
    consts_d = nc.declare_dram_parameter("consts", [128, 8 * WIN], F16,
                                         isOutput=False)
    o_d = nc.declare_dram_parameter("o", [RPC, WIN, NW, D], F16, isOutput=True)

    with tile.TileContext(nc) as tc:
        with (
            tc.tile_pool(name="consts", bufs=1) as consts,
            tc.tile_pool(name="data", bufs=2) as data,
            tc.tile_pool(name="qab", bufs=2) as qabp,
            tc.tile_pool(name="tmp", bufs=1) as tmpp,
            tc.tile_pool(name="vop", bufs=2) as vop,
            tc.tile_pool(name="expp", bufs=5) as expp,
            tc.tile_pool(name="outp", bufs=2) as outp,
            tc.tile_pool(name="recp", bufs=2) as recp,
            tc.tile_pool(name="psim", bufs=2, space="PSUM") as psim,
            tc.tile_pool(name="ppo", bufs=1, space="PSUM") as ppo,
            tc.tile_pool(name="pdn", bufs=1, space="PSUM") as pdn,
        ):
            ctile = consts.tile([128, 8 * WIN], F16, tag="consts")
            nc.sync.dma_start(out=ctile, in_=consts_d[:, :])
            tab = {t: ctile[:, i * WIN:(i + 1) * WIN]
                   for i, t in enumerate(TAB_NAMES)}
            maskt = ctile[:, 6 * WIN:7 * WIN]
            ident = ctile[:, 7 * WIN:8 * WIN]
            ones = consts.tile([WIN, 1], F16, tag="ones")
            nc.vector.memset(ones, 1.0)

            def bcast(t, nwin):
                # [128, WIN] table -> [128, nwin, WIN] window-broadcast view
                return bass.AP(
                    tensor=t.tensor, offset=t.offset,
                    ap=[list(t.ap[0]), [0, nwin], list(t.ap[1])],
                )

            for p in range(NPAIR):
                qp = data.tile([128, N], F16, tag="qp")
                qs = data.tile([128, N], F16, tag="qs")
                kp = data.tile([128, N], F16, tag="kp")
                ks = data.tile([128, N], F16, tag="ks")
                # qAB: slot (c,0)=qrotA win c, slot (c,1)=qrotB win c+1
                qab = qabp.tile([128, NW, 2, WIN], F16, tag="qab")
                ta = tmpp.tile([128, N], F16, tag="ta")
                tb = tmpp.tile([128, N], F16, tag="tb")
                vo0 = vop.tile([WIN, NW, D], F16, tag="vo0")
                vo1 = vop.tile([WIN, NW, D], F16, tag="vo1")
                vo = [vo0, vo1]
                out0 = outp.tile([WIN, NW, D], F16, tag="out0")
                out1 = outp.tile([WIN, NW, D], F16, tag="out1")
                outr = [out0, out1]

                # ---- loads, segment-interleaved; small leading segments
                # so the first QK groups start ~5us instead of ~9
                SEGS = [(0, 1024), (1024, 2048), (2048, 3072), (3072, 4096)]
                for s, (lo, hi) in enumerate(SEGS):
                    sl = slice(lo, hi)
                    nc.sync.dma_start(out=kp[:, sl], in_=kp_d[p][:, sl])
                    nc.sync.dma_start(out=ks[:, sl], in_=ks_d[p][:, sl])
                    nc.sync.dma_start(out=qp[:, sl], in_=qp_d[p][:, sl])
                    nc.sync.dma_start(out=qs[:, sl], in_=qs_d[p][:, sl])
                    if s == 0:
                        nc.sync.dma_start(out=vo[0][:, 0:16, :],
                                          in_=vo_d[2 * p][:, 0:16, :])
                    elif s == 1:
                        nc.sync.dma_start(out=vo[1][:, 0:16, :],
                                          in_=vo_d[2 * p + 1][:, 0:16, :])
                        nc.sync.dma_start(out=vo[0][:, 16:NW, :],
                                          in_=vo_d[2 * p][:, 16:NW, :])
                    elif s == 2:
                        nc.sync.dma_start(out=vo[1][:, 16:NW, :],
                                          in_=vo_d[2 * p + 1][:, 16:NW, :])
                # qB slot of window NW is unused -> zero once
                nc.vector.memset(qab[:, NW - 1, 1, :], 0.0)

                # ---- rotary, segmented; d' on partitions so all ops are
                # full-width.  q -> qab slots (A: angle i vs own chunk,
                # B: angle i+128 vs prev chunk); k in-place into kp.
                # Coarser segmentation than the loads: DVE/Pool per-op
                # overhead is 60-190ns, so fewer, bigger ops win.

                def rot_b(wlo, whi):
                    # qB for windows [wlo, whi) -> slots (w-1, 1).
                    # cos product via tb, sin product direct into the slot,
                    # add on Pool.
                    n = whi - wlo
                    if n <= 0:
                        return
                    csl = slice(wlo * WIN, whi * WIN)
                    tbv = tb[:, csl].rearrange("p (w i) -> p w i", i=WIN)
                    nc.vector.tensor_mul(
                        tbv, qp[:, csl].rearrange("p (w i) -> p w i", i=WIN),
                        bcast(tab["cqb"], n))
                    nc.vector.tensor_mul(
                        qab[:, wlo - 1:whi - 1, 1, :],
                        qs[:, csl].rearrange("p (w i) -> p w i", i=WIN),
                        bcast(tab["sqb"], n))
                    nc.gpsimd.tensor_tensor(
                        out=qab[:, wlo - 1:whi - 1, 1, :],
                        in0=qab[:, wlo - 1:whi - 1, 1, :],
                        in1=tbv, op=mybir.AluOpType.add)

                for s, (lo, hi) in enumerate(SEGS):
                    sl = slice(lo, hi)
                    w0 = lo // WIN
                    WSEG = (hi - lo) // WIN
                    qpv = qp[:, sl].rearrange("p (w i) -> p w i", i=WIN)
                    qsv = qs[:, sl].rearrange("p (w i) -> p w i", i=WIN)
                    # boundary first: slot (w0-1, 1) = qB win w0 unblocks the
                    # previous segment's last QK group as soon as this
                    # segment's loads land (instead of after its full rotary)
                    if s > 0:
                        rot_b(w0, w0 + 1)
                    # k: krot in-place into kp (QK lhsT next-most urgent)
                    kpv = kp[:, sl].rearrange("p (w i) -> p w i", i=WIN)
                    ksv = ks[:, sl].rearrange("p (w i) -> p w i", i=WIN)
                    nc.vector.tensor_mul(kpv, kpv, bcast(tab["ck"], WSEG))
                    nc.gpsimd.tensor_mul(ksv, ksv, bcast(tab["sk"], WSEG))
                    nc.gpsimd.tensor_tensor(out=kpv, in0=kpv, in1=ksv,
                                            op=mybir.AluOpType.add)
                    # A variant: windows w0..w0+WSEG-1 -> slots (w, 0)
                    nc.vector.tensor_mul(
                        ta[:, sl].rearrange("p (w i) -> p w i", i=WIN),
                        qpv, bcast(tab["cqa"], WSEG))
                    nc.vector.tensor_mul(
                        qab[:, w0:w0 + WSEG, 0, :], qsv, bcast(tab["sqa"], WSEG))
                    nc.gpsimd.tensor_tensor(
                        out=qab[:, w0:w0 + WSEG, 0, :],
                        in0=qab[:, w0:w0 + WSEG, 0, :],
                        in1=ta[:, sl].rearrange("p (w i) -> p w i", i=WIN),
                        op=mybir.AluOpType.add)
                    # B variant for this segment's windows (minus boundary)
                    rot_b(w0 + 1 if s > 0 else 1, w0 + WSEG)

                # ---- chunk pipeline, ROW-INTERLEAVED: PE executes its queue
                # in order, so issuing row 1's early groups after row 0's
                # late ones would head-of-line block on late load segments.
                prev_expt = {0: None, 1: None}
                po = {}
                dn = {}
                for g in range(NW // 4):
                    for j in (0, 1):
                        b0 = 64 * j
                        sim = psim.tile([128, 4, 2 * WIN], F32)
                        for cc in range(4):
                            c = 4 * g + cc
                            nc.tensor.matmul(
                                sim[:, cc, :],
                                lhsT=kp[b0:b0 + 64, c * WIN:(c + 1) * WIN],
                                rhs=qab[b0:b0 + 64, c, :, :],
                                start=True, stop=False)
                            nc.tensor.matmul(
                                sim[:, cc, 0:WIN], lhsT=maskt, rhs=ident,
                                start=False, stop=True)
                        expt = expp.tile([128, 4, 2 * WIN], F16, tag="expt")
                        nc.scalar.activation(
                            out=expt, in_=sim,
                            func=mybir.ActivationFunctionType.Exp)
                        if g % 2 == 0:
                            # po slots are exactly 256B so 8 windows fill one
                            # PSUM bank -- a 65-col slot would cross the 2KB
                            # bank boundary and corrupt accumulation.  The
                            # softmax denominator comes from separate
                            # near-free ap_size=1 matmuls into dn.
                            po[j] = ppo.tile([128, 8, D], F32, tag=f"po{j}", name=f"po{j}")
                            dn[j] = pdn.tile([128, 8], F32, tag=f"dn{j}", name=f"dn{j}")
                        for cc in range(4):
                            w = 4 * g + cc
                            slot = po[j][:, w % 8, :]
                            dslot = dn[j][:, w % 8 : w % 8 + 1]
                            own = expt[:, cc, 0:WIN]
                            if w == 0:
                                nc.tensor.matmul(
                                    slot, lhsT=own, rhs=vo[j][:, 0, :],
                                    start=True, stop=True)
                                nc.tensor.matmul(
                                    dslot, lhsT=own, rhs=ones,
                                    start=True, stop=True)
                            else:
                                pt = expt if cc > 0 else prev_expt[j]
                                prev = pt[:, (w - 1) % 4, WIN:2 * WIN]
                                nc.tensor.matmul(
                                    slot, lhsT=prev, rhs=vo[j][:, w - 1, :],
                                    start=True, stop=False)
                                nc.tensor.matmul(
                                    slot, lhsT=own, rhs=vo[j][:, w, :],
                                    start=False, stop=True)
                                nc.tensor.matmul(
                                    dslot, lhsT=prev, rhs=ones,
                                    start=True, stop=False)
                                nc.tensor.matmul(
                                    dslot, lhsT=own, rhs=ones,
                                    start=False, stop=True)
                        prev_expt[j] = expt
                        if g % 2 == 1:
                            w0 = 8 * (g // 2)
                            rec = recp.tile([128, 8], F32, tag="rec")
                            nc.vector.reciprocal(rec, dn[j])
                            rb = bass.AP(
                                tensor=rec.tensor, offset=rec.offset,
                                ap=[list(rec.ap[0]), [1, 8], [0, D]])
                            # GPSIMD cannot read PSUM -> normalize on DVE
                            nc.vector.tensor_tensor(
                                out=outr[j][:, w0:w0 + 8, :],
                                in0=po[j], in1=rb,
                                op=mybir.AluOpType.mult)
                for j in (0, 1):
                    nc.sync.dma_start(out=o_d[2 * p + j][:, 0:16, :],
                                      in_=outr[j][:, 0:16, :])
                for j in (0, 1):
                    nc.sync.dma_start(out=o_d[2 * p + j][:, 16:NW, :],
                                      in_=outr[j][:, 16:NW, :])

    nc.compile()
    return nc


_NC_CACHE = None


def _get_nc():
    global _NC_CACHE
    if _NC_CACHE is None:
        _NC_CACHE = build_bass()
    return _NC_CACHE


def _in_maps(q, k, v):
    q_ = np.asarray(q, dtype=np.float32).reshape(ROWS, N, D)
    k_ = np.asarray(k, dtype=np.float32).reshape(ROWS, N, D)
    v_ = np.asarray(v, dtype=np.float32).reshape(ROWS, N, D)
    tabs = _tables()
    maskt = np.triu(np.full((WIN, WIN), MASKVAL, dtype=np.float32), 1).astype(np.float16)
    ident = np.eye(WIN, dtype=np.float16)
    consts = np.concatenate(
        [tabs[t] for t in TAB_NAMES] + [maskt, ident], axis=1)

    maps = []
    for c in range(NCORES):
        m = {"consts": consts}
        qp = np.empty((NPAIR, 128, N), np.float16)
        qsh = np.empty((NPAIR, 128, N), np.float16)
        kp = np.empty((NPAIR, 128, N), np.float16)
        ksh = np.empty((NPAIR, 128, N), np.float16)
        vo = np.empty((RPC, WIN, NW, D), np.float16)
        for p in range(NPAIR):
            for j in (0, 1):
                r = c * RPC + 2 * p + j
                qdm = q_[r].T            # [64, N]
                kdm = k_[r].T
                sl = slice(64 * j, 64 * j + 64)
                qp[p, sl] = qdm
                qsh[p, sl] = np.roll(qdm, -32, axis=0)
                kp[p, sl] = kdm
                ksh[p, sl] = np.roll(kdm, -32, axis=0)
        for rr in range(RPC):
            r = c * RPC + rr
            vv = v_[r].reshape(NW, WIN, D).transpose(1, 0, 2)  # [WIN, NW, D]
            vo[rr] = vv
        m.update(qp=qp, qs=qsh, kp=kp, ks=ksh, vo=vo)
        maps.append(m)
    return maps


def _run(q, k, v, **kw):
    nc = _get_nc()
    res = run_bass_kernel_spmd(nc, _in_maps(q, k, v), list(range(NCORES)), **kw)
    outs = []
    for c in range(NCORES):
        o = res.results[c]["o"].astype(np.float32)   # [RPC, WIN, NW, D]
        outs.append(o.transpose(0, 2, 1, 3).reshape(RPC, N, D))
    out = np.concatenate(outs, axis=0).reshape(B, H, N, D)
    return np.ascontiguousarray(out), res


def kernel(q, k, v):
    out, _ = _run(q, k, v)
    return out


# revision 5
# speedup vs baseline: 1.0326x; 1.0282x over previous
"""Local (windowed) attention with rotary embeddings on 8 TRN2 NeuronCores, v2.

Problem: B=4 H=8 N=4096 D=64, window=128, look_backward=1 (j=256 keys/window),
rotary (position-in-context), causal+pad mask, softmax, PV.

v2 redesign (vs the transpose-based v1 at 106.7us):
  - d-major layout: q/k stored [d, t] so QK contracts d on PE partitions
    directly -- ZERO PE transposes (v1 spent 41us there).
  - fp16 everywhere off the PSUM path: matmuls run 1 cycle/col (4x over
    fp32), DVE elementwise gets the 2x_1p mode.
  - Rows pair-packed on partitions: rotary DVE/Pool ops run at full width;
    per-row matmul operands use partition-base 0/64 (sim-validated).
  - Rotate-half shift is a partition rotation in d-major, which no
    compute engine can do cheaply -- host supplies pre-shifted copies
    (qs/ks) via DMA instead.
  - Causal mask = one extra fp16 matmul (constant -30000 strict-upper
    matrix x identity) accumulated into the QK PSUM group.
  - Per-window rotary tables (angles i, i+128 for q / i for k) with the
    A/B relative-rotation trick: chunk c's keys serve windows c (own, qA)
    and c+1 (prev, qB).  Tables are [128,128] broadcast over windows.
  - PV in fp16 with a ones-column on v producing the softmax denominator;
    normalize = DVE reciprocal + Pool broadcast-multiply; fp16 output.

Cost-model budget per core (4 rows): DMA ~37us (serial bus, the wall),
ACT exp ~33us, DVE ~28us, PE ~27us, Pool ~27us.
"""

import numpy as np

import concourse.bass as bass
import concourse.bacc as bacc
import concourse.tile as tile
from concourse import mybir
from concourse.bass_utils import run_bass_kernel_spmd

B, H, N, D = 4, 8, 4096, 64
WIN = 128
NW = N // WIN            # 32 windows
NCORES = 8
ROWS = B * H             # 32 packed batch rows
RPC = ROWS // NCORES     # 4 rows per core
NPAIR = RPC // 2         # 2 row-pairs per core
ROPE = 10000.0
SCALE = D ** -0.5        # folded into q tables
MASKVAL = -30000.0
NSEG = 4                 # rotary / load segmentation
SEGC = N // NSEG         # 1024 cols per segment

F32 = mybir.dt.float32
F16 = mybir.dt.float16


def _tables():
    """Per-window rotary tables, [128, WIN] each (d' on partitions)."""
    f = np.arange(32, dtype=np.float64)
    omg = ROPE ** (-f / 32.0)                     # [32]
    i = np.arange(WIN, dtype=np.float64)

    def pair(ang_pos, scale):
        ang = ang_pos[None, :] * omg[:, None]     # [32, WIN]
        c64 = np.concatenate([np.cos(ang), np.cos(ang)], axis=0) * scale
        s64 = np.concatenate([-np.sin(ang), np.sin(ang)], axis=0) * scale
        # duplicate for the two packed rows
        return (np.concatenate([c64, c64], axis=0).astype(np.float16),
                np.concatenate([s64, s64], axis=0).astype(np.float16))

    cqa, sqa = pair(i, SCALE)          # q vs own chunk   (angle i)
    cqb, sqb = pair(i + WIN, SCALE)    # q vs prev chunk  (angle i+128)
    ck, sk = pair(i, 1.0)              # k                (angle jj')
    return dict(cqa=cqa, sqa=sqa, cqb=cqb, sqb=sqb, ck=ck, sk=sk)


TAB_NAMES = ["cqa", "sqa", "cqb", "sqb", "ck", "sk"]


def build_bass():
    nc = bacc.Bacc("TRN2", target_bir_lowering=False)
    qp_d = nc.declare_dram_parameter("qp", [NPAIR, 128, N], F16, isOutput=False)
    qs_d = nc.declare_dram_parameter("qs", [NPAIR, 128, N], F16, isOutput=False)
    kp_d = nc.declare_dram_parameter("kp", [NPAIR, 128, N], F16, isOutput=False)
    ks_d = nc.declare_dram_parameter("ks", [NPAIR, 128, N], F16, isOutput=False)
    vo_d = nc.declare_dram_parameter("vo", [RPC, WIN, NW, D], F16, isOutput=False)
    consts_d = nc.declare_dram_parameter("consts", [128, 8 * WIN], F16,
                                         isOutput=False)
    o_d = nc.declare_dram_parameter("o", [RPC, WIN, NW, D], F16, isOutput=True)

    with tile.TileContext(nc) as tc:
        with (
            tc.tile_pool(name="consts", bufs=1) as consts,
            tc.tile_pool(name="data", bufs=2) as data,
            tc.tile_pool(name="qab", bufs=2) as qabp,
            tc.tile_pool(name="tmp", bufs=1) as tmpp,
            tc.tile_pool(name="vop", bufs=2) as vop,
            tc.tile_pool(name="expp", bufs=5) as expp,
            tc.tile_pool(name="outp", bufs=2) as outp,
            tc.tile_pool(name="recp", bufs=2) as recp,
            tc.tile_pool(name="psim", bufs=2, space="PSUM") as psim,
            tc.tile_pool(name="ppo", bufs=1, space="PSUM") as ppo,
            tc.tile_pool(name="pdn", bufs=1, space="PSUM") as pdn,
        ):
            ctile = consts.tile([128, 8 * WIN], F16, tag="consts")
            nc.sync.dma_start(out=ctile, in_=consts_d[:, :])
            tab = {t: ctile[:, i * WIN:(i + 1) * WIN]
                   for i, t in enumerate(TAB_NAMES)}
            maskt = ctile[:, 6 * WIN:7 * WIN]
            ident = ctile[:, 7 * WIN:8 * WIN]
            ones = consts.tile([WIN, 1], F16, tag="ones")
            nc.vector.memset(ones, 1.0)
            # tiny matmuls start the PE p-state ramp clock ~6us before the
            # first real QK, so those run at full clock instead of 2-4x slow
            warm = pdn.tile([128, 8], F32, tag="dn0", name="warm")
            for _ in range(3):
                nc.tensor.matmul(warm[0:1, 0:1], lhsT=ones[:, 0:1],
                                 rhs=ones[:, 0:1], start=True, stop=True)

            def bcast(t, nwin):
                # [128, WIN] table -> [128, nwin, WIN] window-broadcast view
                return bass.AP(
                    tensor=t.tensor, offset=t.offset,
                    ap=[list(t.ap[0]), [0, nwin], list(t.ap[1])],
                )

            for p in range(NPAIR):
                qp = data.tile([128, N], F16, tag="qp")
                qs = data.tile([128, N], F16, tag="qs")
                kp = data.tile([128, N], F16, tag="kp")
                ks = data.tile([128, N], F16, tag="ks")
                # qAB: slot (c,0)=qrotA win c, slot (c,1)=qrotB win c+1
                qab = qabp.tile([128, NW, 2, WIN], F16, tag="qab")
                ta = tmpp.tile([128, N], F16, tag="ta")
                tb = tmpp.tile([128, N], F16, tag="tb")
                vo0 = vop.tile([WIN, NW, D], F16, tag="vo0")
                vo1 = vop.tile([WIN, NW, D], F16, tag="vo1")
                vo = [vo0, vo1]
                out0 = outp.tile([WIN, NW, D], F16, tag="out0")
                out1 = outp.tile([WIN, NW, D], F16, tag="out1")
                outr = [out0, out1]

                # ---- loads, segment-interleaved; small leading segments
                # so the first QK groups start ~5us instead of ~9
                SEGS = [(0, 1024), (1024, 2048), (2048, 3072), (3072, 4096)]
                for s, (lo, hi) in enumerate(SEGS):
                    sl = slice(lo, hi)
                    nc.sync.dma_start(out=kp[:, sl], in_=kp_d[p][:, sl])
                    nc.sync.dma_start(out=ks[:, sl], in_=ks_d[p][:, sl])
                    nc.sync.dma_start(out=qp[:, sl], in_=qp_d[p][:, sl])
                    nc.sync.dma_start(out=qs[:, sl], in_=qs_d[p][:, sl])
                    if s == 0:
                        nc.sync.dma_start(out=vo[0][:, 0:16, :],
                                          in_=vo_d[2 * p][:, 0:16, :])
                    elif s == 1:
                        nc.sync.dma_start(out=vo[1][:, 0:16, :],
                                          in_=vo_d[2 * p + 1][:, 0:16, :])
                        nc.sync.dma_start(out=vo[0][:, 16:NW, :],
                                          in_=vo_d[2 * p][:, 16:NW, :])
                    elif s == 2:
                        nc.sync.dma_start(out=vo[1][:, 16:NW, :],
                                          in_=vo_d[2 * p + 1][:, 16:NW, :])
                # qB slot of window NW is unused -> zero once
                nc.vector.memset(qab[:, NW - 1, 1, :], 0.0)

                # ---- rotary, segmented; d' on partitions so all ops are
                # full-width.  q -> qab slots (A: angle i vs own chunk,
                # B: angle i+128 vs prev chunk); k in-place into kp.
                # Coarser segmentation than the loads: DVE/Pool per-op
                # overhead is 60-190ns, so fewer, bigger ops win.

                def rot_b(wlo, whi, dve_add=False):
                    # qB for windows [wlo, whi) -> slots (w-1, 1).
                    # cos product via tb, sin product direct into the slot,
                    # add on Pool (or DVE for the critical first segment,
                    # where the serial Pool add-chain gates the first QK).
                    n = whi - wlo
                    if n <= 0:
                        return
                    csl = slice(wlo * WIN, whi * WIN)
                    tbv = tb[:, csl].rearrange("p (w i) -> p w i", i=WIN)
                    nc.vector.tensor_mul(
                        tbv, qp[:, csl].rearrange("p (w i) -> p w i", i=WIN),
                        bcast(tab["cqb"], n))
                    nc.vector.tensor_mul(
                        qab[:, wlo - 1:whi - 1, 1, :],
                        qs[:, csl].rearrange("p (w i) -> p w i", i=WIN),
                        bcast(tab["sqb"], n))
                    eng = nc.vector if dve_add else nc.gpsimd
                    eng.tensor_tensor(
                        out=qab[:, wlo - 1:whi - 1, 1, :],
                        in0=qab[:, wlo - 1:whi - 1, 1, :],
                        in1=tbv, op=mybir.AluOpType.add)

                for s, (lo, hi) in enumerate(SEGS):
                    sl = slice(lo, hi)
                    w0 = lo // WIN
                    WSEG = (hi - lo) // WIN
                    qpv = qp[:, sl].rearrange("p (w i) -> p w i", i=WIN)
                    qsv = qs[:, sl].rearrange("p (w i) -> p w i", i=WIN)
                    # boundary first: slot (w0-1, 1) = qB win w0 unblocks the
                    # previous segment's last QK group as soon as this
                    # segment's loads land (instead of after its full rotary)
                    if s > 0:
                        rot_b(w0, w0 + 1)
                    # k: krot in-place into kp (QK lhsT next-most urgent)
                    kpv = kp[:, sl].rearrange("p (w i) -> p w i", i=WIN)
                    ksv = ks[:, sl].rearrange("p (w i) -> p w i", i=WIN)
                    nc.vector.tensor_mul(kpv, kpv, bcast(tab["ck"], WSEG))
                    nc.gpsimd.tensor_mul(ksv, ksv, bcast(tab["sk"], WSEG))
                    nc.gpsimd.tensor_tensor(out=kpv, in0=kpv, in1=ksv,
                                            op=mybir.AluOpType.add)
                    # A variant: windows w0..w0+WSEG-1 -> slots (w, 0)
                    nc.vector.tensor_mul(
                        ta[:, sl].rearrange("p (w i) -> p w i", i=WIN),
                        qpv, bcast(tab["cqa"], WSEG))
                    nc.vector.tensor_mul(
                        qab[:, w0:w0 + WSEG, 0, :], qsv, bcast(tab["sqa"], WSEG))
                    nc.gpsimd.tensor_tensor(
                        out=qab[:, w0:w0 + WSEG, 0, :],
                        in0=qab[:, w0:w0 + WSEG, 0, :],
                        in1=ta[:, sl].rearrange("p (w i) -> p w i", i=WIN),
                        op=mybir.AluOpType.add)
                    # B variant for this segment's windows (minus boundary)
                    rot_b(w0 + 1 if s > 0 else 1, w0 + WSEG, dve_add=False)

                # ---- chunk pipeline, ROW-INTERLEAVED: PE executes its queue
                # in order, so issuing row 1's early groups after row 0's
                # late ones would head-of-line block on late load segments.
                prev_expt = {0: None, 1: None}
                po = {}
                dn = {}
                for g in range(NW // 4):
                    for j in (0, 1):
                        b0 = 64 * j
                        sim = psim.tile([128, 4, 2 * WIN], F32)
                        for cc in range(4):
                            c = 4 * g + cc
                            nc.tensor.matmul(
                                sim[:, cc, :],
                                lhsT=kp[b0:b0 + 64, c * WIN:(c + 1) * WIN],
                                rhs=qab[b0:b0 + 64, c, :, :],
                                start=True, stop=False)
                            nc.tensor.matmul(
                                sim[:, cc, 0:WIN], lhsT=maskt, rhs=ident,
                                start=False, stop=True)
                        expt = expp.tile([128, 4, 2 * WIN], F16, tag="expt")
                        if g == NW // 4 - 1:
                            # chunk 31's prev-half serves nonexistent window
                            # 32 -- skip its exp (cols 896:1024)
                            ev = bass.AP(tensor=expt.tensor, offset=expt.offset,
                                         ap=[list(expt.ap[0]), [1, 896]])
                            sv = bass.AP(tensor=sim.tensor, offset=sim.offset,
                                         ap=[list(sim.ap[0]), [1, 896]])
                            nc.scalar.activation(
                                out=ev, in_=sv,
                                func=mybir.ActivationFunctionType.Exp)
                        else:
                            nc.scalar.activation(
                                out=expt, in_=sim,
                                func=mybir.ActivationFunctionType.Exp)
                        if g % 2 == 0:
                            # po slots are exactly 256B so 8 windows fill one
                            # PSUM bank -- a 65-col slot would cross the 2KB
                            # bank boundary and corrupt accumulation.  The
                            # softmax denominator comes from separate
                            # near-free ap_size=1 matmuls into dn.
                            po[j] = ppo.tile([128, 8, D], F32, tag=f"po{j}", name=f"po{j}")
                            dn[j] = pdn.tile([128, 8], F32, tag=f"dn{j}", name=f"dn{j}")
                        for cc in range(4):
                            w = 4 * g + cc
                            slot = po[j][:, w % 8, :]
                            dslot = dn[j][:, w % 8 : w % 8 + 1]
                            own = expt[:, cc, 0:WIN]
                            if w == 0:
                                nc.tensor.matmul(
                                    slot, lhsT=own, rhs=vo[j][:, 0, :],
                                    start=True, stop=True)
                                nc.tensor.matmul(
                                    dslot, lhsT=own, rhs=ones,
                                    start=True, stop=True)
                            else:
                                pt = expt if cc > 0 else prev_expt[j]
                                prev = pt[:, (w - 1) % 4, WIN:2 * WIN]
                                nc.tensor.matmul(
                                    slot, lhsT=prev, rhs=vo[j][:, w - 1, :],
                                    start=True, stop=False)
                                nc.tensor.matmul(
                                    slot, lhsT=own, rhs=vo[j][:, w, :],
                                    start=False, stop=True)
                                nc.tensor.matmul(
                                    dslot, lhsT=prev, rhs=ones,
                                    start=True, stop=False)
                                nc.tensor.matmul(
                                    dslot, lhsT=own, rhs=ones,
                                    start=False, stop=True)
                        prev_expt[j] = expt

                        def norm(lo, hi):
                            # normalize po slots [lo,hi) -> windows 4g+..
                            wbase = 8 * (g // 2)
                            rec = recp.tile([128, 8], F32, tag="rec")
                            nc.vector.reciprocal(
                                rec[:, lo:hi], dn[j][:, lo:hi])
                            rb = bass.AP(
                                tensor=rec.tensor,
                                offset=rec.offset + lo,
                                ap=[list(rec.ap[0]), [1, hi - lo], [0, D]])
                            # GPSIMD cannot read PSUM -> normalize on DVE
                            nc.vector.tensor_tensor(
                                out=outr[j][:, wbase + lo:wbase + hi, :],
                                in0=po[j][:, lo:hi, :], in1=rb,
                                op=mybir.AluOpType.mult)

                        last = p == NPAIR - 1
                        if last and g == 6:
                            norm(0, 4)   # windows 24-27: PVs already done
                        elif last and g == 7:
                            norm(4, 8)
                        elif g % 2 == 1:
                            norm(0, 8)
                for j in (0, 1):
                    nc.sync.dma_start(out=o_d[2 * p + j][:, 0:16, :],
                                      in_=outr[j][:, 0:16, :])
                if p < NPAIR - 1:
                    for j in (0, 1):
                        nc.sync.dma_start(out=o_d[2 * p + j][:, 16:NW, :],
                                          in_=outr[j][:, 16:NW, :])
                else:
                    # tail pair: ship 16-27 while windows 28-31 still
                    # normalize, so the critical out-DMA is only 4 windows
                    for j in (0, 1):
                        nc.sync.dma_start(out=o_d[2 * p + j][:, 16:28, :],
                                          in_=outr[j][:, 16:28, :])
                    for j in (0, 1):
                        nc.sync.dma_start(out=o_d[2 * p + j][:, 28:NW, :],
                                          in_=outr[j][:, 28:NW, :])

    nc.compile()
    return nc


_NC_CACHE = None


def _get_nc():
    global _NC_CACHE
    if _NC_CACHE is None:
        _NC_CACHE = build_bass()
    return _NC_CACHE


def _in_maps(q, k, v):
    q_ = np.asarray(q, dtype=np.float32).reshape(ROWS, N, D)
    k_ = np.asarray(k, dtype=np.float32).reshape(ROWS, N, D)
    v_ = np.asarray(v, dtype=np.float32).reshape(ROWS, N, D)
    tabs = _tables()
    maskt = np.triu(np.full((WIN, WIN), MASKVAL, dtype=np.float32), 1).astype(np.float16)
    ident = np.eye(WIN, dtype=np.float16)
    consts = np.concatenate(
        [tabs[t] for t in TAB_NAMES] + [maskt, ident], axis=1)

    maps = []
    for c in range(NCORES):
        m = {"consts": consts}
        qp = np.empty((NPAIR, 128, N), np.float16)
        qsh = np.empty((NPAIR, 128, N), np.float16)
        kp = np.empty((NPAIR, 128, N), np.float16)
        ksh = np.empty((NPAIR, 128, N), np.float16)
        vo = np.empty((RPC, WIN, NW, D), np.float16)
        for p in range(NPAIR):
            for j in (0, 1):
                r = c * RPC + 2 * p + j
                qdm = q_[r].T            # [64, N]
                kdm = k_[r].T
                sl = slice(64 * j, 64 * j + 64)
                qp[p, sl] = qdm
                qsh[p, sl] = np.roll(qdm, -32, axis=0)
                kp[p, sl] = kdm
                ksh[p, sl] = np.roll(kdm, -32, axis=0)
        for rr in range(RPC):
            r = c * RPC + rr
            vv = v_[r].reshape(NW, WIN, D).transpose(1, 0, 2)  # [WIN, NW, D]
            vo[rr] = vv
        m.update(qp=qp, qs=qsh, kp=kp, ks=ksh, vo=vo)
        maps.append(m)
    return maps


def _run(q, k, v, **kw):
    nc = _get_nc()
    res = run_bass_kernel_spmd(nc, _in_maps(q, k, v), list(range(NCORES)), **kw)
    outs = []
    for c in range(NCORES):
        o = res.results[c]["o"].astype(np.float32)   # [RPC, WIN, NW, D]
        outs.append(o.transpose(0, 2, 1, 3).reshape(RPC, N, D))
    out = np.concatenate(outs, axis=0).reshape(B, H, N, D)
    return np.ascontiguousarray(out), res


def kernel(q, k, v):
    out, _ = _run(q, k, v)
    return out


# revision 6
# speedup vs baseline: 1.0326x; 1.0000x over previous
"""Local (windowed) attention with rotary embeddings on 8 TRN2 NeuronCores, v2.

Problem: B=4 H=8 N=4096 D=64, window=128, look_backward=1 (j=256 keys/window),
rotary (position-in-context), causal+pad mask, softmax, PV.

v2 redesign (vs the transpose-based v1 at 106.7us):
  - d-major layout: q/k stored [d, t] so QK contracts d on PE partitions
    directly -- ZERO PE transposes (v1 spent 41us there).
  - fp16 everywhere off the PSUM path: matmuls run 1 cycle/col (4x over
    fp32), DVE elementwise gets the 2x_1p mode.
  - Rows pair-packed on partitions: rotary DVE/Pool ops run at full width;
    per-row matmul operands use partition-base 0/64 (sim-validated).
  - Rotate-half shift is a partition rotation in d-major, which no
    compute engine can do cheaply -- host supplies pre-shifted copies
    (qs/ks) via DMA instead.
  - Causal mask = one extra fp16 matmul (constant -30000 strict-upper
    matrix x identity) accumulated into the QK PSUM group.
  - Per-window rotary tables (angles i, i+128 for q / i for k) with the
    A/B relative-rotation trick: chunk c's keys serve windows c (own, qA)
    and c+1 (prev, qB).  Tables are [128,128] broadcast over windows.
  - PV in fp16 with a ones-column on v producing the softmax denominator;
    normalize = DVE reciprocal + Pool broadcast-multiply; fp16 output.

Cost-model budget per core (4 rows): DMA ~37us (serial bus, the wall),
ACT exp ~33us, DVE ~28us, PE ~27us, Pool ~27us.
"""

import numpy as np

import concourse.bass as bass
import concourse.bacc as bacc
import concourse.tile as tile
from concourse import mybir
from concourse.bass_utils import run_bass_kernel_spmd

B, H, N, D = 4, 8, 4096, 64
WIN = 128
NW = N // WIN            # 32 windows
NCORES = 8
ROWS = B * H             # 32 packed batch rows
RPC = ROWS // NCORES     # 4 rows per core
NPAIR = RPC // 2         # 2 row-pairs per core
ROPE = 10000.0
SCALE = D ** -0.5        # folded into q tables
MASKVAL = -30000.0
NSEG = 4                 # rotary / load segmentation
SEGC = N // NSEG         # 1024 cols per segment

F32 = mybir.dt.float32
F16 = mybir.dt.float16


def _tables():
    """Per-window rotary tables, [128, WIN] each (d' on partitions)."""
    f = np.arange(32, dtype=np.float64)
    omg = ROPE ** (-f / 32.0)                     # [32]
    i = np.arange(WIN, dtype=np.float64)

    def pair(ang_pos, scale):
        ang = ang_pos[None, :] * omg[:, None]     # [32, WIN]
        c64 = np.concatenate([np.cos(ang), np.cos(ang)], axis=0) * scale
        s64 = np.concatenate([-np.sin(ang), np.sin(ang)], axis=0) * scale
        # duplicate for the two packed rows
        return (np.concatenate([c64, c64], axis=0).astype(np.float16),
                np.concatenate([s64, s64], axis=0).astype(np.float16))

    cqa, sqa = pair(i, SCALE)          # q vs own chunk   (angle i)
    cqb, sqb = pair(i + WIN, SCALE)    # q vs prev chunk  (angle i+128)
    ck, sk = pair(i, 1.0)              # k                (angle jj')
    return dict(cqa=cqa, sqa=sqa, cqb=cqb, sqb=sqb, ck=ck, sk=sk)


TAB_NAMES = ["cqa", "sqa", "cqb", "sqb", "ck", "sk"]


def build_bass():
    nc = bacc.Bacc("TRN2", target_bir_lowering=False)
    qp_d = nc.declare_dram_parameter("qp", [NPAIR, 128, N], F16, isOutput=False)
    qs_d = nc.declare_dram_parameter("qs", [NPAIR, 128, N], F16, isOutput=False)
    kp_d = nc.declare_dram_parameter("kp", [NPAIR, 128, N], F16, isOutput=False)
    ks_d = nc.declare_dram_parameter("ks", [NPAIR, 128, N], F16, isOutput=False)
    vo_d = nc.declare_dram_parameter("vo", [RPC, WIN, NW, D], F16, isOutput=False)
    consts_d = nc.declare_dram_parameter("consts", [128, 8 * WIN], F16,
                                         isOutput=False)
    o_d = nc.declare_dram_parameter("o", [RPC, WIN, NW, D], F16, isOutput=True)

    with tile.TileContext(nc) as tc:
        with (
            tc.tile_pool(name="consts", bufs=1) as consts,
            tc.tile_pool(name="data", bufs=2) as data,
            tc.tile_pool(name="qab", bufs=2) as qabp,
            tc.tile_pool(name="tmp", bufs=1) as tmpp,
            tc.tile_pool(name="vop", bufs=2) as vop,
            tc.tile_pool(name="expp", bufs=5) as expp,
            tc.tile_pool(name="outp", bufs=2) as outp,
            tc.tile_pool(name="recp", bufs=2) as recp,
            tc.tile_pool(name="psim", bufs=2, space="PSUM") as psim,
            tc.tile_pool(name="ppo", bufs=1, space="PSUM") as ppo,
            tc.tile_pool(name="pdn", bufs=1, space="PSUM") as pdn,
        ):
            ctile = consts.tile([128, 8 * WIN], F16, tag="consts")
            nc.sync.dma_start(out=ctile, in_=consts_d[:, :])
            tab = {t: ctile[:, i * WIN:(i + 1) * WIN]
                   for i, t in enumerate(TAB_NAMES)}
            maskt = ctile[:, 6 * WIN:7 * WIN]
            ident = ctile[:, 7 * WIN:8 * WIN]
            ones = consts.tile([WIN, 1], F16, tag="ones")
            nc.vector.memset(ones, 1.0)
            # tiny matmuls start the PE p-state ramp clock ~6us before the
            # first real QK, so those run at full clock instead of 2-4x slow
            warm = pdn.tile([128, 8], F32, tag="dn0", name="warm")
            for _ in range(3):
                nc.tensor.matmul(warm[0:1, 0:1], lhsT=ones[:, 0:1],
                                 rhs=ones[:, 0:1], start=True, stop=True)

            def bcast(t, nwin):
                # [128, WIN] table -> [128, nwin, WIN] window-broadcast view
                return bass.AP(
                    tensor=t.tensor, offset=t.offset,
                    ap=[list(t.ap[0]), [0, nwin], list(t.ap[1])],
                )

            for p in range(NPAIR):
                qp = data.tile([128, N], F16, tag="qp")
                qs = data.tile([128, N], F16, tag="qs")
                kp = data.tile([128, N], F16, tag="kp")
                ks = data.tile([128, N], F16, tag="ks")
                # qAB: slot (c,0)=qrotA win c, slot (c,1)=qrotB win c+1
                qab = qabp.tile([128, NW, 2, WIN], F16, tag="qab")
                ta = tmpp.tile([128, N], F16, tag="ta")
                tb = tmpp.tile([128, N], F16, tag="tb")
                vo0 = vop.tile([WIN, NW, D], F16, tag="vo0")
                vo1 = vop.tile([WIN, NW, D], F16, tag="vo1")
                vo = [vo0, vo1]
                out0 = outp.tile([WIN, NW, D], F16, tag="out0")
                out1 = outp.tile([WIN, NW, D], F16, tag="out1")
                outr = [out0, out1]

                # ---- loads, segment-interleaved; small leading segments
                # so the first QK groups start ~5us instead of ~9
                SEGS = [(0, 1024), (1024, 2048), (2048, 3072), (3072, 4096)]
                for s, (lo, hi) in enumerate(SEGS):
                    sl = slice(lo, hi)
                    nc.sync.dma_start(out=kp[:, sl], in_=kp_d[p][:, sl])
                    nc.sync.dma_start(out=ks[:, sl], in_=ks_d[p][:, sl])
                    nc.sync.dma_start(out=qp[:, sl], in_=qp_d[p][:, sl])
                    nc.sync.dma_start(out=qs[:, sl], in_=qs_d[p][:, sl])
                    if s == 0:
                        nc.sync.dma_start(out=vo[0][:, 0:16, :],
                                          in_=vo_d[2 * p][:, 0:16, :])
                    elif s == 1:
                        nc.sync.dma_start(out=vo[1][:, 0:16, :],
                                          in_=vo_d[2 * p + 1][:, 0:16, :])
                        nc.sync.dma_start(out=vo[0][:, 16:NW, :],
                                          in_=vo_d[2 * p][:, 16:NW, :])
                    elif s == 2:
                        nc.sync.dma_start(out=vo[1][:, 16:NW, :],
                                          in_=vo_d[2 * p + 1][:, 16:NW, :])
                # qB slot of window NW is unused -> zero once
                nc.vector.memset(qab[:, NW - 1, 1, :], 0.0)

                # ---- rotary, segmented; d' on partitions so all ops are
                # full-width.  q -> qab slots (A: angle i vs own chunk,
                # B: angle i+128 vs prev chunk); k in-place into kp.
                # Coarser segmentation than the loads: DVE/Pool per-op
                # overhead is 60-190ns, so fewer, bigger ops win.

                def rot_b(wlo, whi, dve_add=False):
                    # qB for windows [wlo, whi) -> slots (w-1, 1).
                    # cos product via tb, sin product direct into the slot,
                    # add on Pool (or DVE for the critical first segment,
                    # where the serial Pool add-chain gates the first QK).
                    n = whi - wlo
                    if n <= 0:
                        return
                    csl = slice(wlo * WIN, whi * WIN)
                    tbv = tb[:, csl].rearrange("p (w i) -> p w i", i=WIN)
                    nc.vector.tensor_mul(
                        tbv, qp[:, csl].rearrange("p (w i) -> p w i", i=WIN),
                        bcast(tab["cqb"], n))
                    nc.vector.tensor_mul(
                        qab[:, wlo - 1:whi - 1, 1, :],
                        qs[:, csl].rearrange("p (w i) -> p w i", i=WIN),
                        bcast(tab["sqb"], n))
                    eng = nc.vector if dve_add else nc.gpsimd
                    eng.tensor_tensor(
                        out=qab[:, wlo - 1:whi - 1, 1, :],
                        in0=qab[:, wlo - 1:whi - 1, 1, :],
                        in1=tbv, op=mybir.AluOpType.add)

                def rot_ka(lo, hi):
                    # k rotation (in-place into kp) + qA rotation -> slots
                    # (w, 0) for the windows covering cols [lo, hi)
                    sl = slice(lo, hi)
                    w0 = lo // WIN
                    nwn = (hi - lo) // WIN
                    qpv = qp[:, sl].rearrange("p (w i) -> p w i", i=WIN)
                    qsv = qs[:, sl].rearrange("p (w i) -> p w i", i=WIN)
                    kpv = kp[:, sl].rearrange("p (w i) -> p w i", i=WIN)
                    ksv = ks[:, sl].rearrange("p (w i) -> p w i", i=WIN)
                    nc.vector.tensor_mul(kpv, kpv, bcast(tab["ck"], nwn))
                    nc.gpsimd.tensor_mul(ksv, ksv, bcast(tab["sk"], nwn))
                    nc.gpsimd.tensor_tensor(out=kpv, in0=kpv, in1=ksv,
                                            op=mybir.AluOpType.add)
                    nc.vector.tensor_mul(
                        ta[:, sl].rearrange("p (w i) -> p w i", i=WIN),
                        qpv, bcast(tab["cqa"], nwn))
                    nc.vector.tensor_mul(
                        qab[:, w0:w0 + nwn, 0, :], qsv, bcast(tab["sqa"], nwn))
                    nc.gpsimd.tensor_tensor(
                        out=qab[:, w0:w0 + nwn, 0, :],
                        in0=qab[:, w0:w0 + nwn, 0, :],
                        in1=ta[:, sl].rearrange("p (w i) -> p w i", i=WIN),
                        op=mybir.AluOpType.add)

                for s, (lo, hi) in enumerate(SEGS):
                    w0 = lo // WIN
                    WSEG = (hi - lo) // WIN
                    # boundary first: slot (w0-1, 1) = qB win w0 unblocks the
                    # previous segment's last QK group as soon as this
                    # segment's loads land (instead of after its full rotary)
                    if s > 0:
                        rot_b(w0, w0 + 1)
                    if p == 0 and s == 0:
                        # first segment of the whole kernel is the critical
                        # path to the first exp: rotate it in two 4-window
                        # halves so QK group 0 starts ~1us earlier
                        rot_ka(0, 512)
                        rot_b(1, 5)
                        rot_ka(512, 1024)
                        rot_b(5, 8)
                    else:
                        rot_ka(lo, hi)
                        # B variant for this segment (minus boundary)
                        rot_b(w0 + 1 if s > 0 else 1, w0 + WSEG,
                              dve_add=False)

                # ---- chunk pipeline, ROW-INTERLEAVED: PE executes its queue
                # in order, so issuing row 1's early groups after row 0's
                # late ones would head-of-line block on late load segments.
                prev_expt = {0: None, 1: None}
                po = {}
                dn = {}
                for g in range(NW // 4):
                    for j in (0, 1):
                        b0 = 64 * j
                        sim = psim.tile([128, 4, 2 * WIN], F32)
                        for cc in range(4):
                            c = 4 * g + cc
                            nc.tensor.matmul(
                                sim[:, cc, :],
                                lhsT=kp[b0:b0 + 64, c * WIN:(c + 1) * WIN],
                                rhs=qab[b0:b0 + 64, c, :, :],
                                start=True, stop=False)
                            nc.tensor.matmul(
                                sim[:, cc, 0:WIN], lhsT=maskt, rhs=ident,
                                start=False, stop=True)
                        expt = expp.tile([128, 4, 2 * WIN], F16, tag="expt")
                        if g == NW // 4 - 1:
                            # chunk 31's prev-half serves nonexistent window
                            # 32 -- skip its exp (cols 896:1024)
                            ev = bass.AP(tensor=expt.tensor, offset=expt.offset,
                                         ap=[list(expt.ap[0]), [1, 896]])
                            sv = bass.AP(tensor=sim.tensor, offset=sim.offset,
                                         ap=[list(sim.ap[0]), [1, 896]])
                            nc.scalar.activation(
                                out=ev, in_=sv,
                                func=mybir.ActivationFunctionType.Exp)
                        else:
                            nc.scalar.activation(
                                out=expt, in_=sim,
                                func=mybir.ActivationFunctionType.Exp)
                        if g % 2 == 0:
                            # po slots are exactly 256B so 8 windows fill one
                            # PSUM bank -- a 65-col slot would cross the 2KB
                            # bank boundary and corrupt accumulation.  The
                            # softmax denominator comes from separate
                            # near-free ap_size=1 matmuls into dn.
                            po[j] = ppo.tile([128, 8, D], F32, tag=f"po{j}", name=f"po{j}")
                            dn[j] = pdn.tile([128, 8], F32, tag=f"dn{j}", name=f"dn{j}")
                        for cc in range(4):
                            w = 4 * g + cc
                            slot = po[j][:, w % 8, :]
                            dslot = dn[j][:, w % 8 : w % 8 + 1]
                            own = expt[:, cc, 0:WIN]
                            if w == 0:
                                nc.tensor.matmul(
                                    slot, lhsT=own, rhs=vo[j][:, 0, :],
                                    start=True, stop=True)
                                nc.tensor.matmul(
                                    dslot, lhsT=own, rhs=ones,
                                    start=True, stop=True)
                            else:
                                pt = expt if cc > 0 else prev_expt[j]
                                prev = pt[:, (w - 1) % 4, WIN:2 * WIN]
                                nc.tensor.matmul(
                                    slot, lhsT=prev, rhs=vo[j][:, w - 1, :],
                                    start=True, stop=False)
                                nc.tensor.matmul(
                                    slot, lhsT=own, rhs=vo[j][:, w, :],
                                    start=False, stop=True)
                                nc.tensor.matmul(
                                    dslot, lhsT=prev, rhs=ones,
                                    start=True, stop=False)
                                nc.tensor.matmul(
                                    dslot, lhsT=own, rhs=ones,
                                    start=False, stop=True)
                        prev_expt[j] = expt

                        def norm(lo, hi):
                            # normalize po slots [lo,hi) -> windows 4g+..
                            wbase = 8 * (g // 2)
                            rec = recp.tile([128, 8], F32, tag="rec")
                            nc.vector.reciprocal(
                                rec[:, lo:hi], dn[j][:, lo:hi])
                            rb = bass.AP(
                                tensor=rec.tensor,
                                offset=rec.offset + lo,
                                ap=[list(rec.ap[0]), [1, hi - lo], [0, D]])
                            # GPSIMD cannot read PSUM -> normalize on DVE
                            nc.vector.tensor_tensor(
                                out=outr[j][:, wbase + lo:wbase + hi, :],
                                in0=po[j][:, lo:hi, :], in1=rb,
                                op=mybir.AluOpType.mult)

                        last = p == NPAIR - 1
                        if last and g == 6:
                            norm(0, 4)   # windows 24-27: PVs already done
                        elif last and g == 7:
                            norm(4, 8)
                        elif g % 2 == 1:
                            norm(0, 8)
                for j in (0, 1):
                    nc.sync.dma_start(out=o_d[2 * p + j][:, 0:16, :],
                                      in_=outr[j][:, 0:16, :])
                if p < NPAIR - 1:
                    for j in (0, 1):
                        nc.sync.dma_start(out=o_d[2 * p + j][:, 16:NW, :],
                                          in_=outr[j][:, 16:NW, :])
                else:
                    # tail pair: ship 16-27 while windows 28-31 still
                    # normalize, so the critical out-DMA is only 4 windows
                    for j in (0, 1):
                        nc.sync.dma_start(out=o_d[2 * p + j][:, 16:28, :],
                                          in_=outr[j][:, 16:28, :])
                    for j in (0, 1):
                        nc.sync.dma_start(out=o_d[2 * p + j][:, 28:NW, :],
                                          in_=outr[j][:, 28:NW, :])

    nc.compile()
    return nc


_NC_CACHE = None


def _get_nc():
    global _NC_CACHE
    if _NC_CACHE is None:
        _NC_CACHE = build_bass()
    return _NC_CACHE


def _in_maps(q, k, v):
    q_ = np.asarray(q, dtype=np.float32).reshape(ROWS, N, D)
    k_ = np.asarray(k, dtype=np.float32).reshape(ROWS, N, D)
    v_ = np.asarray(v, dtype=np.float32).reshape(ROWS, N, D)
    tabs = _tables()
    maskt = np.triu(np.full((WIN, WIN), MASKVAL, dtype=np.float32), 1).astype(np.float16)
    ident = np.eye(WIN, dtype=np.float16)
    consts = np.concatenate(
        [tabs[t] for t in TAB_NAMES] + [maskt, ident], axis=1)

    maps = []
    for c in range(NCORES):
        m = {"consts": consts}
        qp = np.empty((NPAIR, 128, N), np.float16)
        qsh = np.empty((NPAIR, 128, N), np.float16)
        kp = np.empty((NPAIR, 128, N), np.float16)
        ksh = np.empty((NPAIR, 128, N), np.float16)
        vo = np.empty((RPC, WIN, NW, D), np.float16)
        for p in range(NPAIR):
            for j in (0, 1):
                r = c * RPC + 2 * p + j
                qdm = q_[r].T            # [64, N]
                kdm = k_[r].T
                sl = slice(64 * j, 64 * j + 64)
                qp[p, sl] = qdm
                qsh[p, sl] = np.roll(qdm, -32, axis=0)
                kp[p, sl] = kdm
                ksh[p, sl] = np.roll(kdm, -32, axis=0)
        for rr in range(RPC):
            r = c * RPC + rr
            vv = v_[r].reshape(NW, WIN, D).transpose(1, 0, 2)  # [WIN, NW, D]
            vo[rr] = vv
        m.update(qp=qp, qs=qsh, kp=kp, ks=ksh, vo=vo)
        maps.append(m)
    return maps


def _run(q, k, v, **kw):
    nc = _get_nc()
    res = run_bass_kernel_spmd(nc, _in_maps(q, k, v), list(range(NCORES)), **kw)
    outs = []
    for c in range(NCORES):
        o = res.results[c]["o"].astype(np.float32)   # [RPC, WIN, NW, D]
        outs.append(o.transpose(0, 2, 1, 3).reshape(RPC, N, D))
    out = np.concatenate(outs, axis=0).reshape(B, H, N, D)
    return np.ascontiguousarray(out), res


def kernel(q, k, v):
    out, _ = _run(q, k, v)
    return out
